# revision 1
# baseline (speedup 1.0000x reference)
"""Quantized LM head: out[b,s,v] = x[b,s,:] @ (quant_weight * scales[:,None]).T

Strategy (8 NeuronCores, tensor-parallel over vocab):
  - Each core owns a 151936/8 = 18992-row vocab slice.
  - Host prep (numpy): transpose weights to [in_features, vocab_slice] and cast
    int32 -> bf16 (lossless for int8-range values); x -> bf16 [896, 2048].
  - Device: out_tile[tok, voc] = sum_k xT[k,tok].T @ wT[k,voc] accumulated in
    fp32 PSUM over the 7 k-tiles; dequant scales (fp32) applied by DVE during
    the (mandatory) PSUM->SBUF eviction via a partition-broadcast scale tile.
  - Gather: concat vocab slices.
"""

import numpy as np
import ml_dtypes

P = 128
IN_F = 896            # 7 * 128
KT = IN_F // P        # 7 k-tiles
TOK = 2048            # 2 * 1024 tokens
VOCAB = 151936
NCORES = 8
V_CORE = VOCAB // NCORES     # 18992 = 37*512 + 48
VB = 2048                    # vocab block: 4 PSUM banks (f32)
MM_N = 512                   # matmul moving free dim (bf16 allows up to 1024)

_BF16 = ml_dtypes.bfloat16
_CACHE = {}


def _vocab_blocks(v_total, vb, mm_n):
    """Group mm_n-wide (plus one ragged) vocab tiles into blocks of <= vb."""
    tiles = []
    o = 0
    while o < v_total:
        w = min(mm_n, v_total - o)
        tiles.append((o, w))
        o += w
    blocks = []
    cur = []
    for t in tiles:
        if cur and (t[0] + t[1] - cur[0][0]) > vb:
            blocks.append(cur)
            cur = []
        cur.append(t)
    if cur:
        blocks.append(cur)
    return blocks  # list of list[(offset, width)]


def _build_program(tok, v_core, vb, reps=1, mm_n=MM_N, evict="mul", dma_out=True,
                   w_eng="sync", psum_bufs=8, out_bufs=4, w_bufs=3,
                   out_ring="alt", split_k_dma=False, fine_psum=True):
    from concourse import bacc, bass, tile
    import concourse.mybir as mybir

    nc = bacc.Bacc(
        "TRN2",
        target_bir_lowering=False,
        debug=False,
        enable_asserts=True,
        num_devices=NCORES,
    )
    bf16 = mybir.dt.bfloat16
    f32 = mybir.dt.float32

    x_d = nc.dram_tensor("x_t", [IN_F, tok], bf16, kind="ExternalInput")
    w_d = nc.dram_tensor("w_t", [IN_F, v_core], bf16, kind="ExternalInput")
    s_d = nc.dram_tensor("s", [v_core], f32, kind="ExternalInput")
    o_d = nc.dram_tensor("out", [tok, v_core], f32, kind="ExternalOutput")

    tt_n = tok // P
    blocks = _vocab_blocks(v_core, vb, mm_n)

    with tile.TileContext(nc) as tc:
        with (
            tc.tile_pool(name="xp", bufs=1) as xp,
            tc.tile_pool(name="wp", bufs=w_bufs) as wp,
            tc.tile_pool(name="scp", bufs=2) as scp,
            tc.tile_pool(name="op", bufs=out_bufs) as op_,
            tc.tile_pool(name="pp", bufs=psum_bufs, space="PSUM") as pp,
        ):
            w_ap = w_d.ap().rearrange("(a p) v -> p a v", p=P)
            for _ in range(reps):
                x_sb = xp.tile([P, KT, tok], bf16, tag="x")
                x_ap = x_d.ap().rearrange("(a p) t -> p a t", p=P)
                if split_k_dma:
                    for k in range(KT):
                        nc.sync.dma_start(out=x_sb[:, k], in_=x_ap[:, k])
                else:
                    nc.sync.dma_start(out=x_sb[:], in_=x_ap)
                for tiles in blocks:
                    off = tiles[0][0]
                    width = tiles[-1][0] + tiles[-1][1] - off
                    w_sb = wp.tile([P, KT, width], bf16, tag="w")
                    if split_k_dma:
                        for k in range(KT):
                            getattr(nc, w_eng).dma_start(
                                out=w_sb[:, k], in_=w_ap[:, k, off : off + width]
                            )
                    else:
                        getattr(nc, w_eng).dma_start(
                            out=w_sb[:], in_=w_ap[:, :, off : off + width]
                        )
                    sc_sb = scp.tile([P, width], f32, tag="sc")
                    s_slice = s_d.ap()[off : off + width]
                    s_bcast = bass.AP(
                        tensor=s_slice.tensor,
                        offset=s_slice.offset,
                        ap=[[0, P]] + list(s_slice.ap),
                    )
                    nc.gpsimd.dma_start(out=sc_sb[:], in_=s_bcast)
                    for tt in range(tt_n):
                        if fine_psum:
                            # one single-bank PSUM tile per 512-wide vocab
                            # tile: DVE evicts each as soon as its 7 matmuls
                            # land, overlapping the next tile's accumulation.
                            o_sb = op_.tile([P, width], f32, tag="o")
                            for v0, vw in tiles:
                                r = v0 - off
                                ps = pp.tile([P, vw], f32, tag="ps")
                                for k in range(KT):
                                    nc.tensor.matmul(
                                        ps[:, :vw],
                                        x_sb[:, k, tt * P : (tt + 1) * P],
                                        w_sb[:, k, r : r + vw],
                                        start=(k == 0),
                                        stop=(k == KT - 1),
                                    )
                                nc.vector.tensor_mul(
                                    o_sb[:, r : r + vw], ps[:, :vw],
                                    sc_sb[:, r : r + vw],
                                )
                        else:
                            ps = pp.tile([P, width], f32, tag="ps")
                            for k in range(KT):
                                for v0, vw in tiles:
                                    r = v0 - off
                                    nc.tensor.matmul(
                                        ps[:, r : r + vw],
                                        x_sb[:, k, tt * P : (tt + 1) * P],
                                        w_sb[:, k, r : r + vw],
                                        start=(k == 0),
                                        stop=(k == KT - 1),
                                    )
                            if evict == "none":
                                continue
                            o_sb = op_.tile([P, width], f32, tag="o")
                            if evict == "mul":
                                nc.vector.tensor_mul(o_sb[:], ps[:], sc_sb[:])
                            else:
                                nc.vector.tensor_copy(o_sb[:], ps[:])
                        if dma_out:
                            if out_ring == "alt":
                                ring = nc.scalar if tt % 2 == 1 else nc.sync
                            elif out_ring == "rr3":
                                ring = (nc.sync, nc.scalar, nc.gpsimd)[tt % 3]
                            else:
                                ring = nc.sync
                            ring.dma_start(
                                out=o_d.ap()[tt * P : (tt + 1) * P, off : off + width],
                                in_=o_sb[:],
                            )
    nc.compile()
    return nc


def _get_program(reps=1, mm_n=MM_N, vb=VB, **kw):
    key = (TOK, V_CORE, vb, reps, mm_n, tuple(sorted(kw.items())))
    if key not in _CACHE:
        _CACHE[key] = _build_program(TOK, V_CORE, vb, reps, mm_n, **kw)
    return _CACHE[key]


def _prep_inputs(x, quant_weight, scales):
    x = np.asarray(x, dtype=np.float32).reshape(TOK, IN_F)
    xT = np.ascontiguousarray(x.T).astype(_BF16)  # [896, 2048]
    qw = np.asarray(quant_weight)
    sc = np.asarray(scales, dtype=np.float32)
    in_maps = []
    for c in range(NCORES):
        wt = np.ascontiguousarray(qw[c * V_CORE : (c + 1) * V_CORE].T).astype(_BF16)
        sp = np.ascontiguousarray(sc[c * V_CORE : (c + 1) * V_CORE])
        in_maps.append({"x_t": xT, "w_t": wt, "s": sp})
    return in_maps


def _run(x, quant_weight, scales, trace=False):
    from concourse.bass_utils import run_bass_kernel_spmd

    nc = _get_program()
    in_maps = _prep_inputs(x, quant_weight, scales)
    res = run_bass_kernel_spmd(nc, in_maps, core_ids=list(range(NCORES)), trace=trace)
    out = np.concatenate([res.results[c]["out"] for c in range(NCORES)], axis=1)
    return out.reshape(2, TOK // 2, VOCAB).astype(np.float32, copy=False), res


def kernel(x, quant_weight, scales):
    out, _ = _run(x, quant_weight, scales, trace=False)
    return out

